# revision 22
# baseline (speedup 1.0000x reference)
"""2-layer weighted-GCN embedding kernel for 8 Trainium2 NeuronCores.

Strategy (dst-sharded message passing):
  - Nodes are sharded by destination across the 8 cores (12500 each, padded
    to 12544 = 98 * 128).  Each core handles every edge whose dst lands in
    its shard, so the scatter-add is purely local.
  - GCN associativity: conv(x) = (A_hat @ x) @ W^T + b, so we aggregate RAW
    features first and apply the dense transform on the (sharded) aggregate.
  - Per-edge gather of source rows uses the SWDGE dma_gather instruction
    (bf16 rows, 256 B each), round-robined over the 4 SWDGE queues so the
    four Q7 core pairs generate descriptors concurrently.  Indices are
    int16, so each gather chunk covers 25088 rows.
  - Scatter-add is an indicator matmul in [feature, dst] orientation: for
    each block of 128 edge slots, acc[f, dst] += msg-contracted with an
    indicator the DVE builds on the fly: ind[p, j] = (j == dstrel[p]) * w[p]
    with w = edge_weight * dinv[dst] (normalization folded in; the gather
    source is xp = dinv * x and r1 is stored pre-scaled by dinv, so both
    layers share the same indicator values).
  - Self-loop contribution is a per-subtile diag(dinv) matmul accumulated
    into the same PSUM group (lhsT = node-major self rows, rhs = diag).
  - The dense transforms use stationary bf16 weights on [f, dst] tiles;
    relu+bias via scalar activation with per-partition bias; one bf16
    identity-transpose back to node-major per layer.
  - The inter-layer AllGather is split in two subtile-halves so the first
    half exchanges while the second half of layer 0 still computes.  Layer 1
    gathers from the [half][core][subtile-rel] gathered layout (its own
    edge tables, reloaded into the same SBUF between layers).

kernel(**inputs) takes the FULL inputs and returns the FULL [100000, 64]
output; everything (sharding, compile, SPMD run, gather of shards) happens
inside.
"""

import numpy as np
import ml_dtypes

import concourse.bass as bass
import concourse.tile as tile
import concourse.bacc as bacc
from concourse import mybir, bass_utils

BF16 = ml_dtypes.bfloat16

F = 128
HID = 128
ENC = 64
NCORES = 8
SUBW = 128
SUPSZ = 5                      # subtiles per supertile (one gather covers these)


def _set_dims(n):
    global N, SHARD, NSUB, SHARD_PAD, CHUNK, XROWS, NSUP, HALF_T, HALF_ROWS
    N = n
    SHARD = N // NCORES
    NSUB = -(-SHARD // SUBW)           # subtiles per shard
    SHARD_PAD = NSUB * SUBW
    CHUNK = 2 * SHARD_PAD              # rows per gather chunk (< 2**15)
    XROWS = NCORES * SHARD_PAD         # padded node-table rows
    NSUP = -(-NSUB // SUPSZ)
    HALF_T = NSUB // 2                 # subtiles in the first collective half
    HALF_ROWS = HALF_T * SUBW


NCHUNK = 4
_set_dims(100000)

_cache = {}


def _edge_tables(chunkv, localv, dst, wn):
    """Per-core gather indices + per-block indicator metadata for one layer.

    chunkv/localv give each edge's source position in that layer's table
    layout. Cell (chunk, dst subtile) slot ranges are shared across cores
    (max count) so the SPMD program is identical."""
    NCELL = NCHUNK * NSUB
    dev = []
    counts = np.zeros((NCORES, NCELL), np.int64)
    for d in range(NCORES):
        lo, hi = d * SHARD, (d + 1) * SHARD
        m = (dst >= lo) & (dst < hi)
        dl = dst[m] - lo
        t = dl // SUBW
        cid = chunkv[m] * NSUB + t
        order = np.argsort(cid, kind="stable")
        cid_s = cid[order]
        counts[d] = np.bincount(cid_s, minlength=NCELL)
        dev.append((cid_s, localv[m][order].astype(np.int16),
                    (dl % SUBW)[order].astype(np.float32), wn[m][order]))
    nb_cell = -(-counts.max(axis=0) // 128)
    cell_off = np.zeros(NCELL + 1, np.int64)
    np.cumsum(nb_cell * 128, out=cell_off[1:])
    TOT = int(cell_off[-1])
    outs = []
    for d in range(NCORES):
        cid_s, sl, dr, wl = dev[d]
        starts = np.zeros(NCELL + 1, np.int64)
        np.cumsum(counts[d], out=starts[1:])
        rank = np.arange(len(cid_s)) - starts[cid_s]
        pos = cell_off[cid_s] + rank
        f_src = np.zeros(TOT, np.int16)
        f_dr = np.zeros(TOT, np.float32)
        f_w = np.zeros(TOT, np.float32)
        f_src[pos] = sl
        f_dr[pos] = dr
        f_w[pos] = wl
        idx16 = np.ascontiguousarray(np.tile(f_src.reshape(-1, 16).T, (8, 1)))
        nblk = TOT // 128
        dstrel = np.ascontiguousarray(f_dr.reshape(nblk, 128).T)
        wns = np.ascontiguousarray(f_w.reshape(nblk, 128).T)
        outs.append((idx16, dstrel, wns))
    return outs, nb_cell.reshape(NCHUNK, NSUB), cell_off, TOT


def _preprocess(x, edge_index, edge_weight, W1, b1, W2, b2, Wf, bf):
    """All host-side numpy prep: normalization, edge partitioning, layouts."""
    src = np.asarray(edge_index[0], dtype=np.int64)
    dst = np.asarray(edge_index[1], dtype=np.int64)
    w = np.asarray(edge_weight, dtype=np.float32)
    x = np.asarray(x, dtype=np.float32)

    deg = np.bincount(dst, weights=w.astype(np.float64), minlength=N) + 1.0
    dinv = (1.0 / np.sqrt(deg)).astype(np.float32)

    xp = x * dinv[:, None]
    xp_pad = np.zeros((XROWS, F), np.float32)
    for o in range(NCORES):
        xp_pad[o * SHARD_PAD:o * SHARD_PAD + SHARD] = xp[o * SHARD:(o + 1) * SHARD]
    xp_bf = xp_pad.astype(BF16)

    wn = w * dinv[dst]  # indicator value: weight * dinv[dst]

    # layer-0 table: the padded node table in shard-major order
    owner = src // SHARD
    off_in = src - owner * SHARD
    src_pad = owner * SHARD_PAD + off_in
    chunk0 = src_pad // CHUNK
    local0 = src_pad - chunk0 * CHUNK
    tabs0, nb0, offs0, TOT0 = _edge_tables(chunk0, local0, dst, wn)

    # layer-1 table: [half][core][subtile-rel][128] AllGather layout
    t_src = off_in // SUBW
    p_src = off_in % SUBW
    half = (t_src >= HALF_T).astype(np.int64)
    row1 = (half * NCORES * HALF_ROWS + owner * HALF_ROWS
            + (t_src - HALF_T * half) * SUBW + p_src)
    chunk1 = row1 // CHUNK
    local1 = row1 - chunk1 * CHUNK
    tabs1, nb1, offs1, TOT1 = _edge_tables(chunk1, local1, dst, wn)

    per_core = []
    for d in range(NCORES):
        lo = d * SHARD
        dvt = np.ones(SHARD_PAD, np.float32)
        dvt[:SHARD] = dinv[lo:lo + SHARD]
        dinv_t = np.ascontiguousarray(dvt.reshape(NSUB, SUBW).T)  # [128, NSUB]
        per_core.append({
            "idx16_0": tabs0[d][0], "dstrel_0": tabs0[d][1], "wns_0": tabs0[d][2],
            "idx16_1": tabs1[d][0], "dstrel_1": tabs1[d][1], "wns_1": tabs1[d][2],
            "dinv_t": dinv_t,
            "xp_self": xp_bf[d * SHARD_PAD:(d + 1) * SHARD_PAD].copy(),
        })

    shared = {
        "xp_bf": xp_bf,
        "w1t": np.ascontiguousarray(np.asarray(W1, np.float32).T).astype(BF16),
        "w2t": np.ascontiguousarray(np.asarray(W2, np.float32).T).astype(BF16),
        "wft": np.ascontiguousarray(np.asarray(Wf, np.float32).T).astype(BF16),
        "b1c": np.asarray(b1, np.float32).reshape(HID, 1).copy(),
        "b2c": np.asarray(b2, np.float32).reshape(HID, 1).copy(),
        "bfc": np.asarray(bf, np.float32).reshape(ENC, 1).copy(),
        "ident": np.eye(128, dtype=np.float32).astype(BF16),
        "iota": np.broadcast_to(np.arange(128, dtype=np.float32),
                                (128, 128)).copy(),
    }
    return shared, per_core, (nb0, offs0, TOT0), (nb1, offs1, TOT1)


def _build(tab0, tab1):
    """Build the SPMD bass program (identical for all 8 cores)."""
    nb0, offs0, TOT0 = tab0
    nb1, offs1, TOT1 = tab1
    TOTM = max(TOT0, TOT1)
    nc = bacc.Bacc("TRN2", target_bir_lowering=False, debug=False,
                   num_devices=NCORES, num_swdge_queues=4)
    f32 = mybir.dt.float32
    bf16 = mybir.dt.bfloat16

    xp_bf_t = nc.dram_tensor("xp_bf", [XROWS, F], bf16, kind="ExternalInput")
    xp_self_t = nc.dram_tensor("xp_self", [SHARD_PAD, F], bf16, kind="ExternalInput")
    idx0_t = nc.dram_tensor("idx16_0", [128, TOT0 // 16], mybir.dt.int16, kind="ExternalInput")
    ds0_t = nc.dram_tensor("dstrel_0", [128, TOT0 // 128], f32, kind="ExternalInput")
    wn0_t = nc.dram_tensor("wns_0", [128, TOT0 // 128], f32, kind="ExternalInput")
    idx1_t = nc.dram_tensor("idx16_1", [128, TOT1 // 16], mybir.dt.int16, kind="ExternalInput")
    ds1_t = nc.dram_tensor("dstrel_1", [128, TOT1 // 128], f32, kind="ExternalInput")
    wn1_t = nc.dram_tensor("wns_1", [128, TOT1 // 128], f32, kind="ExternalInput")
    iota_t = nc.dram_tensor("iota", [128, 128], f32, kind="ExternalInput")
    dinv_t_t = nc.dram_tensor("dinv_t", [128, NSUB], f32, kind="ExternalInput")
    w1t_t = nc.dram_tensor("w1t", [F, HID], bf16, kind="ExternalInput")
    w2t_t = nc.dram_tensor("w2t", [HID, HID], bf16, kind="ExternalInput")
    wft_t = nc.dram_tensor("wft", [HID, ENC], bf16, kind="ExternalInput")
    b1c_t = nc.dram_tensor("b1c", [HID, 1], f32, kind="ExternalInput")
    b2c_t = nc.dram_tensor("b2c", [HID, 1], f32, kind="ExternalInput")
    bfc_t = nc.dram_tensor("bfc", [ENC, 1], f32, kind="ExternalInput")
    ident_t = nc.dram_tensor("ident", [128, 128], bf16, kind="ExternalInput")
    out_t = nc.dram_tensor("out", [SHARD_PAD, ENC], f32, kind="ExternalOutput")

    blocks0 = [[(c, k) for c in range(NCHUNK) for k in range(int(nb0[c][t]))]
               for t in range(NSUB)]
    blocks1 = [[(c, k) for c in range(NCHUNK) for k in range(int(nb1[c][t]))]
               for t in range(NSUB)]

    with tile.TileContext(nc) as tc:
        with tc.tile_pool(name="const", bufs=1) as cst, \
             tc.tile_pool(name="edata", bufs=1) as edata, \
             tc.tile_pool(name="msgp", bufs=2) as msgp, \
             tc.tile_pool(name="indp", bufs=2) as indp, \
             tc.tile_pool(name="selfp", bufs=3) as selfp, \
             tc.tile_pool(name="accp", bufs=3, space="PSUM") as accp, \
             tc.tile_pool(name="epsp", bufs=3, space="PSUM") as epsp, \
             tc.tile_pool(name="fpp", bufs=2, space="PSUM") as fpp, \
             tc.tile_pool(name="work", bufs=3) as work, \
             tc.tile_pool(name="dram", bufs=1, space="DRAM") as dram:

            # ---- persistent SBUF data (edge tables reloaded per layer) ----
            idx_sb = edata.tile([128, TOTM // 16], mybir.dt.int16)
            ds_sb = edata.tile([128, TOTM // 128], f32)
            wns_sb = edata.tile([128, TOTM // 128], f32)
            nc.sync.dma_start(idx_sb[:, :TOT0 // 16], idx0_t[:])
            nc.sync.dma_start(ds_sb[:, :TOT0 // 128], ds0_t[:])
            nc.sync.dma_start(wns_sb[:, :TOT0 // 128], wn0_t[:])

            iota_sb = cst.tile([128, 128], f32)
            dinv_sb = cst.tile([128, NSUB], f32)
            w1t_sb = cst.tile([F, HID], bf16)
            w2t_sb = cst.tile([HID, HID], bf16)
            wft_sb = cst.tile([HID, ENC], bf16)
            b1c_sb = cst.tile([HID, 1], f32)
            b2c_sb = cst.tile([HID, 1], f32)
            bfc_sb = cst.tile([ENC, 1], f32)
            ident_sb = cst.tile([128, 128], bf16)
            for sb_, t_ in ((iota_sb, iota_t), (dinv_sb, dinv_t_t),
                            (w1t_sb, w1t_t), (w2t_sb, w2t_t),
                            (wft_sb, wft_t), (b1c_sb, b1c_t), (b2c_sb, b2c_t),
                            (bfc_sb, bfc_t), (ident_sb, ident_t)):
                nc.sync.dma_start(sb_[:], t_[:])

            # diag(dinv) per subtile, built once, reused by both layers
            diag_sb = edata.tile([128, NSUB * 128], bf16)
            for t in range(NSUB):
                nc.vector.tensor_scalar(
                    out=diag_sb[:, t * 128:(t + 1) * 128], in0=ident_sb[:],
                    scalar1=dinv_sb[:, t:t + 1], scalar2=None,
                    op0=mybir.AluOpType.mult)

            # own-shard r1' rows (node-major, bf16), filled by layer 0
            r1node_sb = edata.tile([128, NSUB * HID], bf16)

            r1shA = dram.tile([HALF_ROWS, HID], bf16)
            r1shB = dram.tile([HALF_ROWS, HID], bf16)
            r1fullA = dram.tile([NCORES * HALF_ROWS, HID], bf16, addr_space="Shared")
            r1fullB = dram.tile([NCORES * HALF_ROWS, HID], bf16, addr_space="Shared")

            def gather_all(hook, layer):
                """AllGather half h of r1 across cores."""
                sh, full = (r1shA, r1fullA) if hook == 0 else (r1shB, r1fullB)
                nc.gpsimd.collective_compute(
                    "AllGather",
                    mybir.AluOpType.bypass,
                    replica_groups=[list(range(NCORES))],
                    ins=[sh[:].opt()],
                    outs=[full[:].opt()],
                )

            def issue_gather(src_of, offs, s, c):
                t0 = s * SUPSZ
                t1 = min((s + 1) * SUPSZ, NSUB) - 1
                start_slot = int(offs[c * NSUB + t0])
                end_slot = int(offs[c * NSUB + t1 + 1])
                L = end_slot - start_slot
                if L == 0:
                    return None, 0
                msg = msgp.tile([128, L], bf16, tag=f"msg{c}", bufs=2)
                nc.gpsimd.dma_gather(
                    msg[:].rearrange("p (b f) -> p b f", f=128),
                    src_of(c),
                    idx_sb[:, start_slot // 16:end_slot // 16],
                    L, L, 128, elem_step=F,
                    single_packet=False,
                    queue_num=c,
                )
                return msg, start_slot

            def aggregate_layer(src_of, layer, nb, offs, blocks, hook=None,
                                premsgs=None):
                for s in range(NSUP):
                    if hook is not None:
                        hook(s)
                    subs = list(range(s * SUPSZ, min((s + 1) * SUPSZ, NSUB)))
                    msgs = {}
                    starts = {}
                    for c in range(NCHUNK):
                        if premsgs is not None and (s, c) in premsgs:
                            m, st = premsgs[(s, c)]
                        else:
                            m, st = issue_gather(src_of, offs, s, c)
                        if m is None:
                            continue
                        msgs[c] = m
                        starts[c] = st

                    # batched DVE indicator build for this supertile:
                    # ind[p, b, j] = (j == dstrel[p, b]) * wns[p, b]
                    inds = {}
                    bstart = {}
                    for c in range(NCHUNK):
                        if c not in msgs:
                            continue
                        b0 = int(offs[c * NSUB + subs[0]]) // 128
                        b1 = int(offs[c * NSUB + subs[-1] + 1]) // 128
                        nbs = b1 - b0
                        bstart[c] = b0
                        indt = indp.tile([128, nbs * 128], bf16,
                                         tag=f"ind{c}", bufs=2)
                        inds[c] = indt
                        ind3 = indt[:].rearrange("p (b j) -> p b j", j=128)
                        nc.vector.tensor_tensor(
                            out=ind3,
                            in0=iota_sb[:, None, :].broadcast_to([128, nbs, 128]),
                            in1=ds_sb[:, b0:b1, None].broadcast_to([128, nbs, 128]),
                            op=mybir.AluOpType.is_equal)
                        nc.vector.tensor_tensor(
                            out=ind3, in0=ind3,
                            in1=wns_sb[:, b0:b1, None].broadcast_to([128, nbs, 128]),
                            op=mybir.AluOpType.mult)

                    for t in subs:
                        acc = accp.tile([128, 128], f32, tag="acc")
                        for c in range(NCHUNK):
                            nbk = int(nb[c][t])
                            if nbk == 0:
                                continue
                            base = int(offs[c * NSUB + t])
                            for k in range(nbk):
                                lb = base // 128 + k - bstart[c]
                                mloc = (base - starts[c]) // 128 + k
                                nc.tensor.matmul(
                                    acc[:],
                                    lhsT=msgs[c][:, mloc * 128:(mloc + 1) * 128],
                                    rhs=inds[c][:, lb * 128:(lb + 1) * 128],
                                    start=(blocks[t][0] == (c, k)),
                                    stop=False,
                                )

                        # self-loop contribution via diag(dinv) matmul,
                        # closing the accumulation group
                        if layer == 0:
                            self_tl = selfp.tile([128, F], bf16, tag="selftl")
                            nc.sync.dma_start(
                                self_tl[:], xp_self_t[t * 128:(t + 1) * 128, :])
                            self_ap = self_tl[:]
                        else:
                            self_ap = r1node_sb[:, t * HID:(t + 1) * HID]
                        nc.tensor.matmul(
                            acc[:],
                            lhsT=self_ap,
                            rhs=diag_sb[:, t * 128:(t + 1) * 128],
                            start=(len(blocks[t]) == 0),
                            stop=True,
                        )

                        # normalized aggregate [f, dst] -> bf16 SBUF
                        sum_bf = work.tile([128, 128], bf16, tag="sum")
                        nc.scalar.activation(sum_bf[:], acc[:],
                                             mybir.ActivationFunctionType.Copy)

                        wsb = w1t_sb if layer == 0 else w2t_sb
                        z = epsp.tile([128, HID], f32, tag="eps")
                        nc.tensor.matmul(z[:], lhsT=wsb[:], rhs=sum_bf[:],
                                         start=True, stop=True)
                        bcol = b1c_sb if layer == 0 else b2c_sb
                        rt = work.tile([128, HID], bf16, tag="rt")
                        nc.scalar.activation(rt[:], z[:],
                                             mybir.ActivationFunctionType.Relu,
                                             bias=bcol[:])

                        if layer == 0:
                            rp = epsp.tile([128, HID], f32, tag="eps")
                            nc.tensor.matmul(rp[:], lhsT=rt[:], rhs=ident_sb[:],
                                             start=True, stop=True)
                            r1v = r1node_sb[:, t * HID:(t + 1) * HID]
                            nc.scalar.activation(
                                r1v, rp[:],
                                mybir.ActivationFunctionType.Copy,
                                scale=dinv_sb[:, t:t + 1])
                            if t < HALF_T:
                                nc.scalar.dma_start(
                                    r1shA[t * 128:(t + 1) * 128, :], r1v)
                            else:
                                nc.scalar.dma_start(
                                    r1shB[(t - HALF_T) * 128:
                                          (t - HALF_T + 1) * 128, :], r1v)
                        else:
                            fp = fpp.tile([ENC, 128], f32, tag="epsf")
                            nc.tensor.matmul(fp[:], lhsT=wft_sb[:], rhs=rt[:],
                                             start=True, stop=True)
                            fz = work.tile([ENC, 128], bf16, tag="fz")
                            nc.scalar.activation(
                                fz[:], fp[:],
                                mybir.ActivationFunctionType.Identity,
                                bias=bfc_sb[:])
                            op = epsp.tile([128, ENC], f32, tag="eps")
                            nc.tensor.matmul(op[:], lhsT=fz[:],
                                             rhs=ident_sb[:ENC, :ENC],
                                             start=True, stop=True)
                            ob = work.tile([128, ENC], f32, tag="ob")
                            nc.scalar.activation(
                                ob[:], op[:],
                                mybir.ActivationFunctionType.Copy)
                            nc.scalar.dma_start(
                                out_t[t * 128:(t + 1) * 128, :], ob[:])

            # layer 0; the first-half AllGather fires once subtiles
            # 0..HALF_T-1 are written (supertile HALF_T//SUPSZ starts there)
            first_b_sup = -(-HALF_T // SUPSZ)
            def hook0(s):
                if s == first_b_sup:
                    gather_all(0, 0)
            aggregate_layer(lambda c: xp_bf_t[c * CHUNK:(c + 1) * CHUNK, :],
                            0, nb0, offs0, blocks0, hook=hook0)

            # reload edge tables for layer 1 (hidden under the collectives)
            nc.sync.dma_start(idx_sb[:, :TOT1 // 16], idx1_t[:])
            nc.sync.dma_start(ds_sb[:, :TOT1 // 128], ds1_t[:])
            nc.sync.dma_start(wns_sb[:, :TOT1 // 128], wn1_t[:])

            def src1(c):
                if c < 2:
                    return r1fullA[c * CHUNK:(c + 1) * CHUNK, :]
                return r1fullB[(c - 2) * CHUNK:(c - 1) * CHUNK, :]

            # pre-issue layer-1 half-A gathers (they only need collective A)
            # so their desc-gen runs during collective B's barrier+transfer
            premsgs = {}
            for s in (0, 1):
                for c in (0, 1):
                    premsgs[(s, c)] = issue_gather(src1, offs1, s, c)
            gather_all(1, 0)
            aggregate_layer(src1, 1, nb1, offs1, blocks1, premsgs=premsgs)

    nc.compile()
    return nc


def kernel(**inputs):
    shared, per_core, tab0, tab1 = _preprocess(
        inputs["x"], inputs["edge_index"], inputs["edge_weight"],
        inputs["W1"], inputs["b1"], inputs["W2"], inputs["b2"],
        inputs["Wf"], inputs["bf"])

    key = (tab0[2], tab1[2], tab0[0].tobytes(), tab1[0].tobytes())
    if key not in _cache:
        _cache[key] = _build(tab0, tab1)
    nc = _cache[key]

    in_maps = []
    for d in range(NCORES):
        m = dict(shared)
        m.update(per_core[d])
        in_maps.append(m)

    res = bass_utils.run_bass_kernel_spmd(nc, in_maps, core_ids=list(range(NCORES)))
    out = np.concatenate(
        [res.results[d]["out"][:SHARD] for d in range(NCORES)], axis=0)
    return out.astype(np.float32)
